# revision 1
# baseline (speedup 1.0000x reference)
"""Trainium2 Bass kernel for a 2-layer dual-branch GCN (nn_ATACGCN).

reference:
    zs, zu, za = split(z)
    ys = elu(adj @ (elu(zs) @ W0) + b0); ys = elu(adj @ (ys @ W1) + b1)
    yu = elu(adj @ (elu(zu) @ W0) + b0); yu = elu(adj @ (yu @ W1) + b1)
    out = concat(ys, yu, za) @ Wl + bl

Strategy: 1D row-shard of the node dimension across 8 NeuronCores. Both
branches share weights, so they are fused into one 128-wide feature block
(block-diagonal W). Each core streams its [16384, 2048] slab of adj^T (bf16)
from HBM twice (once per GCN layer) and accumulates Y^T = H^T @ adjT in
PSUM. The only cross-core exchange is a 1 MiB/rank AllGather of the layer-1
input features H1 between the two layers.

elu(x) is composed as min(exp(x) - 1, max(x, 0)).
"""

import numpy as np
import ml_dtypes

BF16 = ml_dtypes.bfloat16

# Problem constants (hardcoded per harness contract).
N = 16384      # nodes
D = 64         # per-branch width
OUT = 64       # output width
L = 2          # gcn layers
N_CORES = 8
P = 128        # SBUF partitions
RPC = N // N_CORES          # rows (nodes) per core
KT = N // P                 # contraction tiles
T_PC = RPC // P             # node tiles per core
CW_A = min(1024, N)         # stage-A chunk width (input elu)
CW_Y = min(512, RPC)        # adj-matmul PSUM chunk width
NCH_Y = RPC // CW_Y         # PSUM chunks per core


def build_kernel_body(tc, ins, outs, n_cores=N_CORES, n=N, with_collective=True,
                      adj_dtype="bf16", h_dtype="bf16", use_doublerow=False):
    """Emit the per-core Tile program.

    ins/outs: dicts name -> bass.AP of the DRAM I/O tensors:
      adjt [n, rpc] bf16/fp8, zsut [128, n] f32, zat [64, rpc] bf16,
      wbd [128, 2*128] bf16, wlsu [128, 64] bf16, wlza [64, 64] bf16,
      blr [1, 64] bf16, bias [128, 2] f32  ->  outp [rpc, 64] f32

    adj_dtype="fp8": adjt is float8e4 holding adj.T * n (host-scaled into
    [0,1)); the 1/n is folded back in during the ELU pre-scale. H tiles are
    stored fp8 to match the matmul operand dtype.
    """
    import concourse.mybir as mybir

    nc = tc.nc
    dt = mybir.dt
    f32, bf = dt.float32, dt.bfloat16
    AF = mybir.ActivationFunctionType
    ALU = mybir.AluOpType
    fp8 = adj_dtype == "fp8"
    adt = dt.float8e4 if fp8 else bf
    hdt = dt.float8e4 if h_dtype == "fp8" else bf
    if use_doublerow:
        assert fp8 and h_dtype == "fp8", "DoubleRow needs fp8 operands"
    inv_n = 1.0 / n

    rpc = n // n_cores
    kt = n // P
    t_pc = rpc // P
    cw_a = min(1024, n)
    nch_a = n // cw_a
    cw_y = min(512, rpc)         # matmul out cols (PSUM one-bank limit, fp32)
    nch_y = rpc // cw_y
    rest_g = min(8, kt)          # k-tiles per restage DMA
    n_rest = kt // rest_g

    adjt = ins["adjt"]
    zsut = ins["zsut"]
    zat = ins["zat"]
    wbd, wlsu, wlza = ins["wbd"], ins["wlsu"], ins["wlza"]
    blr, bias = ins["blr"], ins["bias"]
    outp = outs["outp"]

    with (
        tc.tile_pool(name="consts", bufs=1) as consts,
        tc.tile_pool(name="hpool", bufs=1) as hpool,
        tc.tile_pool(name="adjp", bufs=8) as adjp,
        tc.tile_pool(name="zp", bufs=3) as zp,
        tc.tile_pool(name="tmp", bufs=2) as tmp,
        tc.tile_pool(name="xp", bufs=1) as xp,
        tc.tile_pool(name="ps", bufs=1, space="PSUM") as ps,
        tc.tile_pool(name="dram", bufs=1, space="DRAM") as dram,
    ):
        # ---- constants to SBUF ----
        wbd_sb = consts.tile([P, L * P], bf, name="wbd_sb")
        nc.sync.dma_start(out=wbd_sb[:], in_=wbd[:])
        wlsu_sb = consts.tile([P, OUT], f32, name="wlsu_sb")
        nc.sync.dma_start(out=wlsu_sb[:], in_=wlsu[:])
        wlza_sb = consts.tile([D, OUT], f32, name="wlza_sb")
        nc.sync.dma_start(out=wlza_sb[:], in_=wlza[:])
        blr_sb = consts.tile([1, OUT], f32, name="blr_sb")
        nc.sync.dma_start(out=blr_sb[:], in_=blr[:])
        bias_sb = consts.tile([P, L], f32, name="bias_sb")
        nc.sync.dma_start(out=bias_sb[:], in_=bias[:])
        zat_sb = consts.tile([D, rpc], f32, name="zat_sb")
        nc.sync.dma_start(out=zat_sb[:], in_=zat[:])
        ones_sb = consts.tile([1, P], f32, name="ones_sb")
        nc.vector.memset(ones_sb[:], 1.0)

        # ---- stage A: E = elu(zsu), H0 = E @ W0bd (node-major) ----
        h0 = hpool.tile([P, n], hdt, name="h0", tag="h0")
        for ch in range(nch_a):
            zch = zp.tile([P, cw_a], bf, name="zch", tag="zch")
            nc.sync.dma_start(out=zch[:], in_=zsut[:, ch * cw_a:(ch + 1) * cw_a])
            e_t = tmp.tile([P, cw_a], f32, name="e_t", tag="e_t")
            nc.scalar.activation(e_t[:], zch[:], AF.Exp)
            m_t = tmp.tile([P, cw_a], f32, name="m_t", tag="m_t")
            nc.vector.tensor_scalar_max(m_t[:], zch[:], 0.0)
            x0 = xp.tile([P, cw_a], bf, name="x0", tag="x0", bufs=2)
            nc.vector.scalar_tensor_tensor(
                x0[:], e_t[:], -1.0, m_t[:], op0=ALU.add, op1=ALU.min
            )
            # 4 node-tiles per PSUM bank, one 512-wide copy out
            for grp in range(cw_a // (4 * P)):
                ph = ps.tile([P, 4 * P], f32, name="ph", tag="ph", bufs=2)
                for t4 in range(4):
                    t = grp * 4 + t4
                    nc.tensor.matmul(
                        ph[:, t4 * P:(t4 + 1) * P],
                        lhsT=x0[:, t * P:(t + 1) * P], rhs=wbd_sb[:, 0:P],
                        start=True, stop=True,
                    )
                g = ch * (cw_a // P) + grp * 4
                nc.vector.tensor_copy(h0[:, g * P:(g + 4) * P], ph[:])

        h_cur = h0
        for layer in range(L):
            # ---- big matmul: Y^T[feat, local nodes] = H^T @ adjT ----
            psy = [
                ps.tile([P, cw_y], f32, name=f"psy{c}", tag=f"psy{c}", bufs=1)
                for c in range(nch_y)
            ]
            if use_doublerow:
                # 2 k-tiles per step: plane j of the 3D APs is k-tile 2*kb+j.
                av = adjt.rearrange("(kb j p) m -> kb p j m", j=2, p=P)
                hv = h_cur.rearrange("p (kb j f) -> p kb j f", j=2, f=P)
                for kb in range(kt // 2):
                    slab = adjp.tile([P, 2, rpc], adt, name="slab", tag="slab")
                    nc.sync.dma_start(out=slab[:], in_=av[kb])
                    for c in range(nch_y):
                        nc.tensor.matmul(
                            psy[c][:],
                            lhsT=hv[:, kb],
                            rhs=slab[:, :, c * cw_y:(c + 1) * cw_y],
                            start=(kb == 0), stop=(kb == kt // 2 - 1),
                            perf_mode=mybir.MatmulPerfMode.DoubleRow,
                        )
            else:
                for k in range(kt):
                    slab = adjp.tile([P, rpc], adt, name="slab", tag="slab")
                    nc.sync.dma_start(out=slab[:], in_=adjt[k * P:(k + 1) * P, :])
                    for c in range(nch_y):
                        nc.tensor.matmul(
                            psy[c][:],
                            lhsT=h_cur[:, k * P:(k + 1) * P],
                            rhs=slab[:, c * cw_y:(c + 1) * cw_y],
                            start=(k == 0), stop=(k == kt - 1),
                        )

            # ---- X^T = elu(Y^T + b) (feature-major, bf16) ----
            xdt = f32 if layer == L - 1 else bf
            xT = xp.tile([P, rpc], xdt, name="xT", tag=f"xT{layer}")
            b_ap = bias_sb[:, layer:layer + 1]
            for c in range(nch_y):
                e_t = tmp.tile([P, cw_y], f32, name="e_t", tag="e_t")
                m_t = tmp.tile([P, cw_y], f32, name="m_t", tag="m_t")
                if fp8:
                    # Y = psy / n ; s = Y + b
                    s_t = tmp.tile([P, cw_y], f32, name="s_t", tag="s_t")
                    nc.vector.tensor_scalar(
                        s_t[:], psy[c][:], inv_n, b_ap, op0=ALU.mult, op1=ALU.add
                    )
                    nc.scalar.activation(e_t[:], s_t[:], AF.Exp)
                    nc.vector.tensor_scalar_max(m_t[:], s_t[:], 0.0)
                else:
                    nc.scalar.activation(e_t[:], psy[c][:], AF.Exp, bias=b_ap)
                    nc.vector.tensor_scalar(
                        m_t[:], psy[c][:], b_ap, 0.0, op0=ALU.add, op1=ALU.max
                    )
                nc.vector.scalar_tensor_tensor(
                    xT[:, c * cw_y:(c + 1) * cw_y], e_t[:], -1.0, m_t[:],
                    op0=ALU.add, op1=ALU.min,
                )

            if layer < L - 1:
                # ---- H1_m = X1 @ W1bd (node-major tiles), AllGather, restage ----
                h1m = xp.tile([P, rpc], hdt, name="h1m", tag="h1m")
                g4 = 4 if t_pc % 4 == 0 else 1
                for grp in range(t_pc // g4):
                    ph = ps.tile([P, 4 * P], f32, name="ph", tag="ph", bufs=2)
                    for t4 in range(g4):
                        t = grp * g4 + t4
                        nc.tensor.matmul(
                            ph[:, t4 * P:(t4 + 1) * P],
                            lhsT=xT[:, t * P:(t + 1) * P],
                            rhs=wbd_sb[:, P:2 * P], start=True, stop=True,
                        )
                    nc.vector.tensor_copy(
                        h1m[:, grp * g4 * P:(grp * g4 + g4) * P],
                        ph[:, :g4 * P])

                h1 = hpool.tile([P, n], hdt, name="h1", tag="h1")
                if with_collective and n_cores > 1:
                    g_in = dram.tile([rpc, P], hdt, name="g_in")
                    nc.sync.dma_start(
                        out=g_in.rearrange("(t p) f -> p t f", p=P),
                        in_=h1m.rearrange("p (t f) -> p t f", f=P),
                    )
                    g_out = dram.tile([n, P], hdt, name="g_out", addr_space="Shared")
                    nc.gpsimd.collective_compute(
                        "AllGather",
                        mybir.AluOpType.bypass,
                        replica_groups=[list(range(n_cores))],
                        ins=[g_in.opt()],
                        outs=[g_out.opt()],
                    )
                    gview = g_out.rearrange("(k p) f -> p k f", p=P)
                    h1view = h1.rearrange("p (k f) -> p k f", f=P)
                    for g in range(n_rest):
                        nc.sync.dma_start(
                            out=h1view[:, g * rest_g:(g + 1) * rest_g, :],
                            in_=gview[:, g * rest_g:(g + 1) * rest_g, :],
                        )
                else:
                    # cost-model-only path (TimelineSim): emit the same DMA
                    # pattern as the collective path, minus the collective.
                    # Numerically invalid for k-tiles of other cores.
                    g_in = dram.tile([rpc, P], hdt, name="g_in")
                    nc.sync.dma_start(
                        out=g_in.rearrange("(t p) f -> p t f", p=P),
                        in_=h1m.rearrange("p (t f) -> p t f", f=P),
                    )
                    g_out = dram.tile([n, P], hdt, name="g_out")
                    nc.sync.dma_start(
                        out=g_out[:rpc, :], in_=g_in[:],
                    )
                    gview = g_out.rearrange("(k p) f -> p k f", p=P)
                    h1view = h1.rearrange("p (k f) -> p k f", f=P)
                    for g in range(n_rest):
                        nc.sync.dma_start(
                            out=h1view[:, g * rest_g:(g + 1) * rest_g, :],
                            in_=gview[:, g * rest_g:(g + 1) * rest_g, :],
                        )
                h_cur = h1
            else:
                # ---- final: out = [ys yu] @ Wl[:128] + za @ Wl[128:] + bl ----
                for t in range(t_pc):
                    po = ps.tile([P, OUT], f32, name="po", tag="po", bufs=2)
                    nc.tensor.matmul(
                        po[:], lhsT=xT[:, t * P:(t + 1) * P], rhs=wlsu_sb[:],
                        start=True, stop=False, skip_group_check=True,
                    )
                    nc.tensor.matmul(
                        po[:], lhsT=zat_sb[:, t * P:(t + 1) * P], rhs=wlza_sb[:],
                        start=False, stop=False, skip_group_check=True,
                    )
                    nc.tensor.matmul(
                        po[:], lhsT=ones_sb[:], rhs=blr_sb[:],
                        start=False, stop=True, skip_group_check=True,
                    )
                    ot = zp.tile([P, OUT], f32, name="ot", tag="ot", bufs=2)
                    nc.vector.tensor_copy(ot[:], po[:])
                    nc.sync.dma_start(out=outp[t * P:(t + 1) * P, :], in_=ot[:])


def build_full(n_cores=N_CORES, n=N, adj_dtype="bf16", h_dtype="bf16",
               use_doublerow=False, num_devices=None, with_collective=True,
               repeats=1):
    """Build + compile the full SPMD Bass module (one program, 8 cores)."""
    import concourse.bacc as bacc
    import concourse.mybir as mybir
    import concourse.tile as tile

    dt = mybir.dt
    f32, bf = dt.float32, dt.bfloat16
    adt = dt.float8e4 if adj_dtype == "fp8" else bf
    rpc = n // n_cores
    if num_devices is None:
        num_devices = n_cores

    nc = bacc.Bacc("TRN2", target_bir_lowering=False, debug=False,
                   num_devices=num_devices)
    ins = {
        "adjt": nc.dram_tensor("adjt", [n, rpc], adt, kind="ExternalInput").ap(),
        "zsut": nc.dram_tensor("zsut", [P, n], bf, kind="ExternalInput").ap(),
        "zat": nc.dram_tensor("zat", [D, rpc], f32, kind="ExternalInput").ap(),
        "wbd": nc.dram_tensor("wbd", [P, L * P], bf, kind="ExternalInput").ap(),
        "wlsu": nc.dram_tensor("wlsu", [P, OUT], f32, kind="ExternalInput").ap(),
        "wlza": nc.dram_tensor("wlza", [D, OUT], f32, kind="ExternalInput").ap(),
        "blr": nc.dram_tensor("blr", [1, OUT], f32, kind="ExternalInput").ap(),
        "bias": nc.dram_tensor("bias", [P, L], f32, kind="ExternalInput").ap(),
    }
    outs = {
        "outp": nc.dram_tensor("outp", [rpc, OUT], f32, kind="ExternalOutput").ap(),
    }
    with tile.TileContext(nc) as tc:
        for _ in range(repeats):
            build_kernel_body(tc, ins, outs, n_cores=n_cores, n=n,
                              adj_dtype=adj_dtype, h_dtype=h_dtype,
                              use_doublerow=use_doublerow,
                              with_collective=with_collective)
    nc.compile()
    return nc


def prep_inputs(z, adj, Ws, bs, Wl, bl, n_cores=N_CORES, n=N, adj_dtype="bf16"):
    """Host-side sharding: build the per-core input maps."""
    rpc = n // n_cores
    z = np.asarray(z, dtype=np.float32)
    adj = np.asarray(adj, dtype=np.float32)
    Ws = np.asarray(Ws, dtype=np.float32)
    bs = np.asarray(bs, dtype=np.float32)
    Wl = np.asarray(Wl, dtype=np.float32)
    bl = np.asarray(bl, dtype=np.float32)

    if adj_dtype == "fp8":
        FP8 = ml_dtypes.float8_e4m3
        adjt = (adj.T * np.float32(n)).astype(FP8)       # [n, n] in [0,1)
    else:
        adjt = np.ascontiguousarray(adj.T).astype(BF16)  # [n, n]
    zsut = np.ascontiguousarray(z[:, :2 * D].T).astype(BF16)  # [128, n]
    zat = np.ascontiguousarray(z[:, 2 * D:].T)               # [64, n] f32

    wbd = np.zeros((P, L * P), dtype=np.float32)
    for l in range(L):
        wbd[:D, l * P:l * P + D] = Ws[l]
        wbd[D:, l * P + D:(l + 1) * P] = Ws[l]
    wbd = wbd.astype(BF16)
    bias = np.stack([np.concatenate([bs[l], bs[l]]) for l in range(L)],
                    axis=1).astype(np.float32)           # [128, L]
    wlsu = np.ascontiguousarray(Wl[:2 * D])
    wlza = np.ascontiguousarray(Wl[2 * D:])
    blr = np.ascontiguousarray(bl.reshape(1, OUT))

    in_maps = []
    for m in range(n_cores):
        in_maps.append({
            "adjt": np.ascontiguousarray(adjt[:, m * rpc:(m + 1) * rpc]),
            "zsut": zsut,
            "zat": np.ascontiguousarray(zat[:, m * rpc:(m + 1) * rpc]),
            "wbd": wbd,
            "wlsu": wlsu,
            "wlza": wlza,
            "blr": blr,
            "bias": bias,
        })
    return in_maps


_NC_CACHE = {}
ADJ_DTYPE = "fp8"
H_DTYPE = "fp8"
USE_DOUBLEROW = True


def kernel(z, adj, Ws, bs, Wl, bl):
    """Full-input entry point: shard, run on 8 NeuronCores, gather."""
    from concourse.bass_utils import run_bass_kernel_spmd

    if "nc" not in _NC_CACHE:
        _NC_CACHE["nc"] = build_full(adj_dtype=ADJ_DTYPE, h_dtype=H_DTYPE,
                                     use_doublerow=USE_DOUBLEROW)
    nc = _NC_CACHE["nc"]

    in_maps = prep_inputs(z, adj, Ws, bs, Wl, bl, adj_dtype=ADJ_DTYPE)
    res = run_bass_kernel_spmd(nc, in_maps, core_ids=list(range(N_CORES)))
    out = np.concatenate(
        [res.results[m]["outp"] for m in range(N_CORES)], axis=0
    ).astype(np.float32)
    return out



# revision 12
# speedup vs baseline: 1.0850x; 1.0850x over previous
"""Trainium2 Bass kernel for a 2-layer dual-branch GCN (nn_ATACGCN).

reference:
    zs, zu, za = split(z)
    ys = elu(adj @ (elu(zs) @ W0) + b0); ys = elu(adj @ (ys @ W1) + b1)
    yu = elu(adj @ (elu(zu) @ W0) + b0); yu = elu(adj @ (yu @ W1) + b1)
    out = concat(ys, yu, za) @ Wl + bl

Strategy: 1D row-shard of the node dimension across 8 NeuronCores. Both
branches share weights, so they are fused into one 128-wide feature block
(block-diagonal W). Each core computes Y^T = H^T @ adjT in PSUM, streaming
its [16384, 2048] slab of adj^T (fp8, host-scaled by N) from HBM.

The kernel is DMA-bound (adj streaming), so the first B_RES kb-blocks of the
adj slab are kept RESIDENT in SBUF across both GCN layers -- layer 2 only
re-streams the non-resident remainder. Stage A (input elu + H0 = E @ W0) is
sharded across cores and AllGathered, like the inter-layer exchange, instead
of being computed redundantly on every core. h-path DMAs ride the ACT HWDGE
ring so they never queue behind bulk adj DMAs on the SP ring.

elu(x) is composed as min(exp(x) - 1, max(x, 0)); exp/relu run on ACT with
the fp8 descale (1/n) and layer bias folded into the activation's
scale/bias operands.
"""

import numpy as np
import ml_dtypes

BF16 = ml_dtypes.bfloat16
FP8 = ml_dtypes.float8_e4m3

# Problem constants (hardcoded per harness contract).
N = 16384      # nodes
D = 64         # per-branch width
OUT = 64       # output width
L = 2          # gcn layers
N_CORES = 8
P = 128        # SBUF partitions
RPC = N // N_CORES          # rows (nodes) per core

# Tunables
B_RES = 32                  # resident kb-blocks (of kt//2 = 64); 4 KiB/part each
RES_GRP = 2                 # kb-blocks per resident group DMA (1 MiB DMAs)
RING1 = 5                   # L1 streaming slab ring depth (512 KiB each)
RING2 = 4                   # L2 streaming slab ring depth (fresh tag: prefetch
                            # across the inter-layer collective)
CW_SUB = 512                # stage-A / elu subchunk width
REST_G = 8                  # k-tiles per restage DMA (256 KiB)


def build_kernel_body(tc, ins, outs, n_cores=N_CORES, n=N, with_collective=True,
                      b_res=B_RES, ring1=RING1, ring2=RING2):
    """Emit the per-core Tile program (fp8 adj + fp8 H, DoubleRow matmuls).

    ins/outs: dicts name -> bass.AP of the DRAM I/O tensors:
      adjt [n, rpc] fp8 (adj.T * n, host-scaled), zsut [128, rpc] bf16,
      zat [64, rpc] bf16, wbd [128, 2*128] bf16, wlsu [128, 64] bf16,
      wlza [64, 64] bf16, blr [1, 64] bf16, bias [128, 2] f32
      ->  outp [rpc, 64] f32
    """
    import concourse.mybir as mybir

    nc = tc.nc
    dt = mybir.dt
    f32, bf = dt.float32, dt.bfloat16
    AF = mybir.ActivationFunctionType
    ALU = mybir.AluOpType
    adt = dt.float8e4
    hdt = dt.float8e4
    inv_n = 1.0 / n

    rpc = n // n_cores
    kt = n // P                  # 128 k-tiles
    nkb = kt // 2                # 64 kb-blocks (DoubleRow: 2 k-tiles each)
    t_pc = rpc // P              # 16 node tiles per core
    cw_y = 512                   # PSUM chunk width (one f32 bank)
    nch_y = rpc // cw_y          # 4
    n_sub = rpc // CW_SUB        # stage-A subchunks
    assert b_res % RES_GRP == 0
    n_res_grp = b_res // RES_GRP

    adjt = ins["adjt"]
    zsut = ins["zsut"]
    zat = ins["zat"]
    wbd, wlsu, wlza = ins["wbd"], ins["wlsu"], ins["wlza"]
    blr, bias = ins["blr"], ins["bias"]
    outp = outs["outp"]

    # DRAM view: q = global k-tile index (0..127).
    adjq = adjt.rearrange("(q p) m -> p q m", p=P)   # [128, 128, rpc]

    with (
        tc.tile_pool(name="consts", bufs=1) as consts,
        tc.tile_pool(name="respool", bufs=1) as respool,
        tc.tile_pool(name="hpool", bufs=1) as hpool,
        tc.tile_pool(name="adjp", bufs=ring1) as adjp,
        tc.tile_pool(name="adjp2", bufs=ring2) as adjp2,
        tc.tile_pool(name="tmp", bufs=2) as tmp,
        tc.tile_pool(name="xp", bufs=1) as xp,
        tc.tile_pool(name="ps", bufs=1, space="PSUM") as ps,
        tc.tile_pool(name="dram", bufs=1, space="DRAM") as dram,
    ):
        # ---- constants to SBUF ----
        wbd_sb = consts.tile([P, L * P], bf, name="wbd_sb")
        nc.scalar.dma_start(out=wbd_sb[:], in_=wbd[:])
        wlsu_sb = consts.tile([P, OUT], bf, name="wlsu_sb")
        nc.scalar.dma_start(out=wlsu_sb[:], in_=wlsu[:])
        wlza_sb = consts.tile([D, OUT], bf, name="wlza_sb")
        nc.scalar.dma_start(out=wlza_sb[:], in_=wlza[:])
        blr_sb = consts.tile([1, OUT], bf, name="blr_sb")
        nc.scalar.dma_start(out=blr_sb[:], in_=blr[:])
        bias_sb = consts.tile([P, L], f32, name="bias_sb")
        nc.scalar.dma_start(out=bias_sb[:], in_=bias[:])
        zat_sb = consts.tile([D, rpc], bf, name="zat_sb")
        nc.sync.dma_start(out=zat_sb[:], in_=zat[:])
        ones_sb = consts.tile([1, P], bf, name="ones_sb")
        nc.vector.memset(ones_sb[:], 1.0)

        # Persistent H tile (shared between layers; 16 KiB/partition).
        # Layout: h[p, q*128 + f] = H[node q*128+p, f].
        h = hpool.tile([P, n], hdt, name="h", tag="h")
        hq = h.rearrange("p (q f) -> p q f", f=P)

        def emit_gather(hm, layer, rest_order=None):
            """AllGather local node-block features hm [P, rpc] -> full h.

            g_in rows are ordered (partition, tile) so that both the g_in
            write and the h restage run with >=2 KiB-contiguous descriptors
            per partition (128 B descriptors otherwise -- far below the
            512 B full-rate SDMA minimum).
            """
            g_in = dram.tile([rpc, P], hdt, name=f"g_in{layer}")
            nc.scalar.dma_start(
                out=g_in.rearrange("(p t) f -> p (t f)", p=P),
                in_=hm[:],
            )
            if with_collective and n_cores > 1:
                g_out = dram.tile([n, P], hdt, name=f"g_out{layer}",
                                  addr_space="Shared")
                nc.gpsimd.collective_compute(
                    "AllGather",
                    mybir.AluOpType.bypass,
                    replica_groups=[list(range(n_cores))],
                    ins=[g_in.opt()],
                    outs=[g_out.opt()],
                )
            else:
                # cost-model-only path (TimelineSim): same DMA pattern minus
                # the collective. Numerically invalid for other cores' tiles.
                g_out = dram.tile([n, P], hdt, name=f"g_out{layer}")
                nc.scalar.dma_start(out=g_out[:rpc, :], in_=g_in[:])
            # g_out row m*rpc + p*t_pc + t holds H[node m*rpc + t*128 + p, :].
            # h free-dim block m*2048 + t*128 + f <- g_out[m, p] run (t f).
            gm = g_out.rearrange("(m p w) f -> p m (w f)", m=n_cores, p=P)
            hm_view = h.rearrange("p (m w) -> p m w", m=n_cores)
            for g in (rest_order or range(n_cores)):
                nc.scalar.dma_start(
                    out=hm_view[:, g:g + 1, :],
                    in_=gm[:, g:g + 1, :],
                )

        # ---- stage A (sharded): E = elu(own zsu), H0_own = E @ W0bd ----
        hm0 = xp.tile([P, rpc], hdt, name="hm", tag="hm")
        for sc in range(n_sub):
            sl = slice(sc * CW_SUB, (sc + 1) * CW_SUB)
            zch = tmp.tile([P, CW_SUB], bf, name="zch", tag="zch")
            nc.scalar.dma_start(out=zch[:], in_=zsut[:, sl])
            e_t = tmp.tile([P, CW_SUB], f32, name="e_t", tag="e_t")
            nc.scalar.activation(e_t[:], zch[:], AF.Exp)
            m_t = tmp.tile([P, CW_SUB], f32, name="m_t", tag="m_t")
            nc.scalar.activation(m_t[:], zch[:], AF.Relu)
            x0 = tmp.tile([P, CW_SUB], bf, name="x0", tag="x0")
            nc.vector.scalar_tensor_tensor(
                x0[:], e_t[:], -1.0, m_t[:], op0=ALU.add, op1=ALU.min
            )
            ph = ps.tile([P, CW_SUB], f32, name="ph", tag="ph", bufs=2)
            for t4 in range(CW_SUB // P):
                nc.tensor.matmul(
                    ph[:, t4 * P:(t4 + 1) * P],
                    lhsT=x0[:, t4 * P:(t4 + 1) * P], rhs=wbd_sb[:, 0:P],
                    start=True, stop=True,
                )
            nc.vector.tensor_copy(hm0[:, sl], ph[:])
        emit_gather(hm0, 0)

        # ---- resident adj groups (loaded once, used by both layers) ----
        res_tiles = [
            respool.tile([P, RES_GRP * 2, rpc], adt, name=f"res{g}",
                         tag=f"res{g}")
            for g in range(n_res_grp)
        ]
        # L1's last ring1 slabs stay valid in SBUF; L2 reuses them directly.
        l1_tail = {}     # kb -> slab tile

        # L2 accumulation order: the ring2-prefetched streamed kbs first
        # (release stream slots the moment h lands), then the remaining
        # streamed kbs Bresenham-interleaved with no-DMA kbs (residents +
        # reused L1-tail slabs) so PE slot-release tracks DMA pace and the
        # resident work fills the DMA-bound phase instead of a serial tail.
        s_kbs = list(range(b_res, nkb - ring1))          # need fresh DMA in L2
        r_kbs = list(range(b_res)) + list(range(nkb - ring1, nkb))
        l2_order = s_kbs[:ring2]
        s_rest, acc = s_kbs[ring2:], 0.0
        ratio = len(r_kbs) / max(1, len(s_rest))
        ri = 0
        for kb in s_rest:
            l2_order.append(kb)
            acc += ratio
            while acc >= 1.0 and ri < len(r_kbs):
                l2_order.append(r_kbs[ri])
                ri += 1
                acc -= 1.0
        l2_order.extend(r_kbs[ri:])
        assert sorted(l2_order) == list(range(nkb))
        # restage-DMA order: deliver h q-groups in L2 consumption order
        rest2, seen = [], set()
        for kb in l2_order:
            g = (2 * kb) // REST_G
            if g not in seen:
                seen.add(g)
                rest2.append(g)
        kb_orders = [list(range(nkb)), l2_order]

        for layer in range(L):
            # ---- big matmul: Y^T[feat, local nodes] = H^T @ adjT ----
            psy = [
                ps.tile([P, cw_y], f32, name=f"psy{c}", tag=f"psy{c}", bufs=1)
                for c in range(nch_y)
            ]
            order = kb_orders[layer]
            for pos, kb in enumerate(order):
                if kb < b_res:
                    g, i = divmod(kb, RES_GRP)
                    if layer == 0 and i == 0:
                        nc.sync.dma_start(
                            out=res_tiles[g][:],
                            in_=adjq[:, g * RES_GRP * 2:(g + 1) * RES_GRP * 2, :],
                        )
                    rhs3 = res_tiles[g][:, 2 * i:2 * i + 2, :]
                elif layer == 1 and kb in l1_tail:
                    rhs3 = l1_tail[kb][:, :, :]
                else:
                    pool = adjp if layer == 0 else adjp2
                    slab = pool.tile([P, 2, rpc], adt, name="slab",
                                     tag=f"slab{layer}")
                    nc.sync.dma_start(
                        out=slab[:], in_=adjq[:, 2 * kb:2 * kb + 2, :]
                    )
                    if layer == 0 and kb >= nkb - ring1:
                        l1_tail[kb] = slab
                    rhs3 = slab[:, :, :]
                lhsT = hq[:, 2 * kb:2 * kb + 2, :]
                for c in range(nch_y):
                    nc.tensor.matmul(
                        psy[c][:],
                        lhsT=lhsT,
                        rhs=rhs3[:, :, c * cw_y:(c + 1) * cw_y],
                        start=(pos == 0), stop=(pos == nkb - 1),
                        perf_mode=mybir.MatmulPerfMode.DoubleRow,
                    )

            # ---- per-chunk: X^T = elu(Y^T/n + b), then H1 | final out ----
            b_ap = bias_sb[:, layer:layer + 1]
            hm1 = None
            if layer < L - 1:
                hm1 = xp.tile([P, rpc], hdt, name="hm", tag="hm")
            for c in range(nch_y):
                e_t = tmp.tile([P, cw_y], f32, name="e_t", tag="e_t")
                nc.scalar.activation(e_t[:], psy[c][:], AF.Exp,
                                     bias=b_ap, scale=inv_n)
                m_t = tmp.tile([P, cw_y], f32, name="m_t", tag="m_t")
                nc.scalar.activation(m_t[:], psy[c][:], AF.Relu,
                                     bias=b_ap, scale=inv_n)
                xc = tmp.tile([P, cw_y], bf, name="xc", tag="xc")
                nc.vector.scalar_tensor_tensor(
                    xc[:], e_t[:], -1.0, m_t[:], op0=ALU.add, op1=ALU.min,
                )
                if layer < L - 1:
                    # H1 chunk: 4 node tiles -> one PSUM bank -> hm1
                    ph = ps.tile([P, 4 * P], f32, name="ph", tag="ph", bufs=2)
                    for t4 in range(4):
                        nc.tensor.matmul(
                            ph[:, t4 * P:(t4 + 1) * P],
                            lhsT=xc[:, t4 * P:(t4 + 1) * P],
                            rhs=wbd_sb[:, P:2 * P], start=True, stop=True,
                        )
                    nc.vector.tensor_copy(
                        hm1[:, c * cw_y:(c + 1) * cw_y], ph[:])
                else:
                    # final: out = [ys yu] @ Wl[:128] + za @ Wl[128:] + bl
                    po = ps.tile([P, 4 * OUT], f32, name="po", tag="po",
                                 bufs=2)
                    for t4 in range(4):
                        t = c * 4 + t4
                        osl = slice(t4 * OUT, (t4 + 1) * OUT)
                        nc.tensor.matmul(
                            po[:, osl], lhsT=xc[:, t4 * P:(t4 + 1) * P],
                            rhs=wlsu_sb[:],
                            start=True, stop=False, skip_group_check=True,
                        )
                        nc.tensor.matmul(
                            po[:, osl], lhsT=zat_sb[:, t * P:(t + 1) * P],
                            rhs=wlza_sb[:],
                            start=False, stop=False, skip_group_check=True,
                        )
                        nc.tensor.matmul(
                            po[:, osl], lhsT=ones_sb[:], rhs=blr_sb[:],
                            start=False, stop=True, skip_group_check=True,
                        )
                    ot = tmp.tile([P, 4 * OUT], f32, name="ot", tag="ot")
                    nc.vector.tensor_copy(ot[:], po[:])
                    nc.sync.dma_start(
                        out=outp[c * cw_y:(c + 1) * cw_y, :].rearrange(
                            "(t p) f -> p t f", p=P),
                        in_=ot.rearrange("p (t f) -> p t f", f=OUT),
                    )
            if layer < L - 1:
                emit_gather(hm1, 1, rest_order=rest2)


def build_full(n_cores=N_CORES, n=N, adj_dtype="fp8", h_dtype="fp8",
               use_doublerow=True, num_devices=None, with_collective=True,
               repeats=1, b_res=B_RES, ring1=RING1, ring2=RING2):
    """Build + compile the full SPMD Bass module (one program, 8 cores)."""
    import concourse.bacc as bacc
    import concourse.mybir as mybir
    import concourse.tile as tile

    dt = mybir.dt
    f32, bf = dt.float32, dt.bfloat16
    adt = dt.float8e4
    rpc = n // n_cores
    if num_devices is None:
        num_devices = n_cores

    nc = bacc.Bacc("TRN2", target_bir_lowering=False, debug=False,
                   num_devices=num_devices)
    ins = {
        "adjt": nc.dram_tensor("adjt", [n, rpc], adt, kind="ExternalInput").ap(),
        "zsut": nc.dram_tensor("zsut", [P, rpc], bf, kind="ExternalInput").ap(),
        "zat": nc.dram_tensor("zat", [D, rpc], bf, kind="ExternalInput").ap(),
        "wbd": nc.dram_tensor("wbd", [P, L * P], bf, kind="ExternalInput").ap(),
        "wlsu": nc.dram_tensor("wlsu", [P, OUT], bf, kind="ExternalInput").ap(),
        "wlza": nc.dram_tensor("wlza", [D, OUT], bf, kind="ExternalInput").ap(),
        "blr": nc.dram_tensor("blr", [1, OUT], bf, kind="ExternalInput").ap(),
        "bias": nc.dram_tensor("bias", [P, L], f32, kind="ExternalInput").ap(),
    }
    outs = {
        "outp": nc.dram_tensor("outp", [rpc, OUT], f32, kind="ExternalOutput").ap(),
    }
    with tile.TileContext(nc) as tc:
        for _ in range(repeats):
            build_kernel_body(tc, ins, outs, n_cores=n_cores, n=n,
                              with_collective=with_collective, b_res=b_res,
                              ring1=ring1, ring2=ring2)
    nc.compile()
    return nc


def prep_inputs(z, adj, Ws, bs, Wl, bl, n_cores=N_CORES, n=N, adj_dtype="fp8"):
    """Host-side sharding: build the per-core input maps."""
    rpc = n // n_cores
    z = np.asarray(z, dtype=np.float32)
    adj = np.asarray(adj, dtype=np.float32)
    Ws = np.asarray(Ws, dtype=np.float32)
    bs = np.asarray(bs, dtype=np.float32)
    Wl = np.asarray(Wl, dtype=np.float32)
    bl = np.asarray(bl, dtype=np.float32)

    adjt = (adj.T * np.float32(n)).astype(FP8)           # [n, n] in [0,1)
    zsut = np.ascontiguousarray(z[:, :2 * D].T).astype(BF16)  # [128, n]
    zat = np.ascontiguousarray(z[:, 2 * D:].T).astype(BF16)   # [64, n]

    wbd = np.zeros((P, L * P), dtype=np.float32)
    for l in range(L):
        wbd[:D, l * P:l * P + D] = Ws[l]
        wbd[D:, l * P + D:(l + 1) * P] = Ws[l]
    wbd = wbd.astype(BF16)
    bias = np.stack([np.concatenate([bs[l], bs[l]]) for l in range(L)],
                    axis=1).astype(np.float32)           # [128, L]
    wlsu = np.ascontiguousarray(Wl[:2 * D]).astype(BF16)
    wlza = np.ascontiguousarray(Wl[2 * D:]).astype(BF16)
    blr = np.ascontiguousarray(bl.reshape(1, OUT)).astype(BF16)

    in_maps = []
    for m in range(n_cores):
        sl = slice(m * rpc, (m + 1) * rpc)
        in_maps.append({
            "adjt": np.ascontiguousarray(adjt[:, sl]),
            "zsut": np.ascontiguousarray(zsut[:, sl]),
            "zat": np.ascontiguousarray(zat[:, sl]),
            "wbd": wbd,
            "wlsu": wlsu,
            "wlza": wlza,
            "blr": blr,
            "bias": bias,
        })
    return in_maps


_NC_CACHE = {}
ADJ_DTYPE = "fp8"
H_DTYPE = "fp8"
USE_DOUBLEROW = True


def kernel(z, adj, Ws, bs, Wl, bl):
    """Full-input entry point: shard, run on 8 NeuronCores, gather."""
    from concourse.bass_utils import run_bass_kernel_spmd

    if "nc" not in _NC_CACHE:
        _NC_CACHE["nc"] = build_full()
    nc = _NC_CACHE["nc"]

    in_maps = prep_inputs(z, adj, Ws, bs, Wl, bl)
    res = run_bass_kernel_spmd(nc, in_maps, core_ids=list(range(N_CORES)))
    out = np.concatenate(
        [res.results[m]["outp"] for m in range(N_CORES)], axis=0
    ).astype(np.float32)
    return out


# revision 13
# speedup vs baseline: 1.1477x; 1.0578x over previous
"""Trainium2 Bass kernel for a 2-layer dual-branch GCN (nn_ATACGCN).

reference:
    zs, zu, za = split(z)
    ys = elu(adj @ (elu(zs) @ W0) + b0); ys = elu(adj @ (ys @ W1) + b1)
    yu = elu(adj @ (elu(zu) @ W0) + b0); yu = elu(adj @ (yu @ W1) + b1)
    out = concat(ys, yu, za) @ Wl + bl

Strategy: 1D row-shard of the node dimension across 8 NeuronCores. Both
branches share weights, so they are fused into one 128-wide feature block
(block-diagonal W). Each core computes Y^T = H^T @ adjT in PSUM, streaming
its [16384, 2048] slab of adj^T (fp8, host-scaled by N) from HBM.

The kernel is DMA-bound (adj streaming), so the first B_RES kb-blocks of the
adj slab are kept RESIDENT in SBUF across both GCN layers -- layer 2 only
re-streams the non-resident remainder. Stage A (input elu + H0 = E @ W0) is
sharded across cores and AllGathered, like the inter-layer exchange, instead
of being computed redundantly on every core. h-path DMAs ride the ACT HWDGE
ring so they never queue behind bulk adj DMAs on the SP ring.

elu(x) is composed as min(exp(x) - 1, max(x, 0)); exp/relu run on ACT with
the fp8 descale (1/n) and layer bias folded into the activation's
scale/bias operands.
"""

import numpy as np
import ml_dtypes

BF16 = ml_dtypes.bfloat16
FP8 = ml_dtypes.float8_e4m3

# Problem constants (hardcoded per harness contract).
N = 16384      # nodes
D = 64         # per-branch width
OUT = 64       # output width
L = 2          # gcn layers
N_CORES = 8
P = 128        # SBUF partitions
RPC = N // N_CORES          # rows (nodes) per core

# Tunables
B_RES = 32                  # resident kb-blocks (of kt//2 = 64); 4 KiB/part each
RES_GRP = 2                 # kb-blocks per resident group DMA (1 MiB DMAs)
RING1 = 5                   # L1 streaming slab ring depth (512 KiB each)
RING2 = 4                   # L2 streaming slab ring depth (fresh tag: prefetch
                            # across the inter-layer collective)
CW_SUB = 512                # stage-A / elu subchunk width
REST_G = 8                  # k-tiles per restage DMA (256 KiB)


def build_kernel_body(tc, ins, outs, n_cores=N_CORES, n=N, with_collective=True,
                      b_res=B_RES, ring1=RING1, ring2=RING2):
    """Emit the per-core Tile program (fp8 adj + fp8 H, DoubleRow matmuls).

    ins/outs: dicts name -> bass.AP of the DRAM I/O tensors:
      adjt [n, rpc] fp8 (adj.T * n, host-scaled), zsut [128, rpc] bf16,
      zat [64, rpc] bf16, wbd [128, 2*128] bf16, wlsu [128, 64] bf16,
      wlza [64, 64] bf16, blr [1, 64] bf16, bias [128, 2] f32
      ->  outp [rpc, 64] f32
    """
    import concourse.mybir as mybir

    nc = tc.nc
    dt = mybir.dt
    f32, bf = dt.float32, dt.bfloat16
    AF = mybir.ActivationFunctionType
    ALU = mybir.AluOpType
    adt = dt.float8e4
    hdt = dt.float8e4
    inv_n = 1.0 / n

    rpc = n // n_cores
    kt = n // P                  # 128 k-tiles
    nkb = kt // 2                # 64 kb-blocks (DoubleRow: 2 k-tiles each)
    t_pc = rpc // P              # 16 node tiles per core
    cw_y = 512                   # PSUM chunk width (one f32 bank)
    nch_y = rpc // cw_y          # 4
    n_sub = rpc // CW_SUB        # stage-A subchunks
    assert b_res % RES_GRP == 0
    n_res_grp = b_res // RES_GRP

    adjt = ins["adjt"]
    zsut = ins["zsut"]
    zat = ins["zat"]
    wbd, wlsu, wlza = ins["wbd"], ins["wlsu"], ins["wlza"]
    blr, bias = ins["blr"], ins["bias"]
    outp = outs["outp"]

    # DRAM view: q = global k-tile index (0..127).
    adjq = adjt.rearrange("(q p) m -> p q m", p=P)   # [128, 128, rpc]

    with (
        tc.tile_pool(name="consts", bufs=1) as consts,
        tc.tile_pool(name="respool", bufs=1) as respool,
        tc.tile_pool(name="hpool", bufs=1) as hpool,
        tc.tile_pool(name="adjp", bufs=ring1) as adjp,
        tc.tile_pool(name="adjp2", bufs=ring2) as adjp2,
        tc.tile_pool(name="tmp", bufs=2) as tmp,
        tc.tile_pool(name="xp", bufs=1) as xp,
        tc.tile_pool(name="ps", bufs=1, space="PSUM") as ps,
        tc.tile_pool(name="dram", bufs=1, space="DRAM") as dram,
    ):
        # ---- constants to SBUF ----
        wbd_sb = consts.tile([P, L * P], bf, name="wbd_sb")
        nc.scalar.dma_start(out=wbd_sb[:], in_=wbd[:])
        wlsu_sb = consts.tile([P, OUT], bf, name="wlsu_sb")
        nc.scalar.dma_start(out=wlsu_sb[:], in_=wlsu[:])
        wlza_sb = consts.tile([D, OUT], bf, name="wlza_sb")
        nc.scalar.dma_start(out=wlza_sb[:], in_=wlza[:])
        blr_sb = consts.tile([1, OUT], bf, name="blr_sb")
        nc.scalar.dma_start(out=blr_sb[:], in_=blr[:])
        bias_sb = consts.tile([P, L], f32, name="bias_sb")
        nc.scalar.dma_start(out=bias_sb[:], in_=bias[:])
        zat_sb = consts.tile([D, rpc], bf, name="zat_sb")
        nc.sync.dma_start(out=zat_sb[:], in_=zat[:])
        ones_sb = consts.tile([1, P], bf, name="ones_sb")
        nc.vector.memset(ones_sb[:], 1.0)

        # Persistent H tile (shared between layers; 16 KiB/partition).
        # Layout: h[p, q*128 + f] = H[node q*128+p, f].
        h = hpool.tile([P, n], hdt, name="h", tag="h")
        hq = h.rearrange("p (q f) -> p q f", f=P)

        def emit_gather(hm, layer, rest_order=None):
            """AllGather local node-block features hm [P, rpc] -> full h.

            g_in rows are ordered (partition, tile) so that both the g_in
            write and the h restage run with >=2 KiB-contiguous descriptors
            per partition (128 B descriptors otherwise -- far below the
            512 B full-rate SDMA minimum).
            """
            g_in = dram.tile([rpc, P], hdt, name=f"g_in{layer}")
            nc.scalar.dma_start(
                out=g_in.rearrange("(p t) f -> p (t f)", p=P),
                in_=hm[:],
            )
            if with_collective and n_cores > 1:
                g_out = dram.tile([n, P], hdt, name=f"g_out{layer}",
                                  addr_space="Shared")
                nc.gpsimd.collective_compute(
                    "AllGather",
                    mybir.AluOpType.bypass,
                    replica_groups=[list(range(n_cores))],
                    ins=[g_in.opt()],
                    outs=[g_out.opt()],
                )
            else:
                # cost-model-only path (TimelineSim): same DMA pattern minus
                # the collective. Numerically invalid for other cores' tiles.
                g_out = dram.tile([n, P], hdt, name=f"g_out{layer}")
                nc.scalar.dma_start(out=g_out[:rpc, :], in_=g_in[:])
            # g_out row m*rpc + p*t_pc + t holds H[node m*rpc + t*128 + p, :].
            # h free-dim block m*2048 + t*128 + f <- g_out[m, p] run (t f).
            gm = g_out.rearrange("(m p w) f -> p m (w f)", m=n_cores, p=P)
            hm_view = h.rearrange("p (m w) -> p m w", m=n_cores)
            for g in (rest_order or range(n_cores)):
                nc.scalar.dma_start(
                    out=hm_view[:, g:g + 1, :],
                    in_=gm[:, g:g + 1, :],
                )

        # ---- stage A (sharded): E = elu(own zsu), H0_own = E @ W0bd ----
        hm0 = xp.tile([P, rpc], hdt, name="hm", tag="hm")
        for sc in range(n_sub):
            sl = slice(sc * CW_SUB, (sc + 1) * CW_SUB)
            zch = tmp.tile([P, CW_SUB], bf, name="zch", tag="zch")
            nc.scalar.dma_start(out=zch[:], in_=zsut[:, sl])
            e_t = tmp.tile([P, CW_SUB], f32, name="e_t", tag="e_t")
            nc.scalar.activation(e_t[:], zch[:], AF.Exp)
            m_t = tmp.tile([P, CW_SUB], f32, name="m_t", tag="m_t")
            nc.scalar.activation(m_t[:], zch[:], AF.Relu)
            x0 = tmp.tile([P, CW_SUB], bf, name="x0", tag="x0")
            nc.vector.scalar_tensor_tensor(
                x0[:], e_t[:], -1.0, m_t[:], op0=ALU.add, op1=ALU.min
            )
            ph = ps.tile([P, CW_SUB], f32, name="ph", tag="ph", bufs=2)
            for t4 in range(CW_SUB // P):
                nc.tensor.matmul(
                    ph[:, t4 * P:(t4 + 1) * P],
                    lhsT=x0[:, t4 * P:(t4 + 1) * P], rhs=wbd_sb[:, 0:P],
                    start=True, stop=True,
                )
            nc.vector.tensor_copy(hm0[:, sl], ph[:])
        emit_gather(hm0, 0)

        # ---- resident adj groups (loaded once, used by both layers) ----
        res_tiles = [
            respool.tile([P, RES_GRP * 2, rpc], adt, name=f"res{g}",
                         tag=f"res{g}")
            for g in range(n_res_grp)
        ]
        # L1's last ring1 slabs stay valid in SBUF; L2 reuses them directly.
        l1_tail = {}     # kb -> slab tile

        # L2 accumulation order: the ring2-prefetched streamed kbs first
        # (release stream slots the moment h lands), then the remaining
        # streamed kbs Bresenham-interleaved with no-DMA kbs (residents +
        # reused L1-tail slabs) so PE slot-release tracks DMA pace and the
        # resident work fills the DMA-bound phase instead of a serial tail.
        s_kbs = list(range(b_res, nkb - ring1))          # need fresh DMA in L2
        r_kbs = list(range(b_res)) + list(range(nkb - ring1, nkb))
        l2_order = s_kbs[:ring2]
        s_rest, acc = s_kbs[ring2:], 0.0
        ratio = len(r_kbs) / max(1, len(s_rest))
        ri = 0
        for kb in s_rest:
            l2_order.append(kb)
            acc += ratio
            while acc >= 1.0 and ri < len(r_kbs):
                l2_order.append(r_kbs[ri])
                ri += 1
                acc -= 1.0
        l2_order.extend(r_kbs[ri:])
        assert sorted(l2_order) == list(range(nkb))
        # restage-DMA order: deliver h m-groups in L2 consumption order
        rest2, seen = [], set()
        for kb in l2_order:
            g = kb // (nkb // n_cores)
            if g not in seen:
                seen.add(g)
                rest2.append(g)
        kb_orders = [list(range(nkb)), l2_order]

        for layer in range(L):
            # ---- big matmul: Y^T[feat, local nodes] = H^T @ adjT ----
            psy = [
                ps.tile([P, cw_y], f32, name=f"psy{c}", tag=f"psy{c}", bufs=1)
                for c in range(nch_y)
            ]
            order = kb_orders[layer]
            for pos, kb in enumerate(order):
                if kb < b_res:
                    g, i = divmod(kb, RES_GRP)
                    if layer == 0 and i == 0:
                        nc.sync.dma_start(
                            out=res_tiles[g][:],
                            in_=adjq[:, g * RES_GRP * 2:(g + 1) * RES_GRP * 2, :],
                        )
                    rhs3 = res_tiles[g][:, 2 * i:2 * i + 2, :]
                elif layer == 1 and kb in l1_tail:
                    rhs3 = l1_tail[kb][:, :, :]
                else:
                    pool = adjp if layer == 0 else adjp2
                    slab = pool.tile([P, 2, rpc], adt, name="slab",
                                     tag=f"slab{layer}")
                    nc.sync.dma_start(
                        out=slab[:], in_=adjq[:, 2 * kb:2 * kb + 2, :]
                    )
                    if layer == 0 and kb >= nkb - ring1:
                        l1_tail[kb] = slab
                    rhs3 = slab[:, :, :]
                lhsT = hq[:, 2 * kb:2 * kb + 2, :]
                for c in range(nch_y):
                    nc.tensor.matmul(
                        psy[c][:],
                        lhsT=lhsT,
                        rhs=rhs3[:, :, c * cw_y:(c + 1) * cw_y],
                        start=(pos == 0), stop=(pos == nkb - 1),
                        perf_mode=mybir.MatmulPerfMode.DoubleRow,
                    )

            # ---- per-chunk: X^T = elu(Y^T/n + b), then H1 | final out ----
            b_ap = bias_sb[:, layer:layer + 1]
            hm1 = None
            if layer < L - 1:
                hm1 = xp.tile([P, rpc], hdt, name="hm", tag="hm")
            for c in range(nch_y):
                e_t = tmp.tile([P, cw_y], f32, name="e_t", tag="e_t")
                nc.scalar.activation(e_t[:], psy[c][:], AF.Exp,
                                     bias=b_ap, scale=inv_n)
                m_t = tmp.tile([P, cw_y], f32, name="m_t", tag="m_t")
                nc.scalar.activation(m_t[:], psy[c][:], AF.Relu,
                                     bias=b_ap, scale=inv_n)
                xc = tmp.tile([P, cw_y], bf, name="xc", tag="xc")
                nc.vector.scalar_tensor_tensor(
                    xc[:], e_t[:], -1.0, m_t[:], op0=ALU.add, op1=ALU.min,
                )
                if layer < L - 1:
                    # H1 chunk: 4 node tiles -> one PSUM bank -> hm1
                    ph = ps.tile([P, 4 * P], f32, name="ph", tag="ph", bufs=2)
                    for t4 in range(4):
                        nc.tensor.matmul(
                            ph[:, t4 * P:(t4 + 1) * P],
                            lhsT=xc[:, t4 * P:(t4 + 1) * P],
                            rhs=wbd_sb[:, P:2 * P], start=True, stop=True,
                        )
                    nc.vector.tensor_copy(
                        hm1[:, c * cw_y:(c + 1) * cw_y], ph[:])
                else:
                    # final: out = [ys yu] @ Wl[:128] + za @ Wl[128:] + bl
                    po = ps.tile([P, 4 * OUT], f32, name="po", tag="po",
                                 bufs=2)
                    for t4 in range(4):
                        t = c * 4 + t4
                        osl = slice(t4 * OUT, (t4 + 1) * OUT)
                        nc.tensor.matmul(
                            po[:, osl], lhsT=xc[:, t4 * P:(t4 + 1) * P],
                            rhs=wlsu_sb[:],
                            start=True, stop=False, skip_group_check=True,
                        )
                        nc.tensor.matmul(
                            po[:, osl], lhsT=zat_sb[:, t * P:(t + 1) * P],
                            rhs=wlza_sb[:],
                            start=False, stop=False, skip_group_check=True,
                        )
                        nc.tensor.matmul(
                            po[:, osl], lhsT=ones_sb[:], rhs=blr_sb[:],
                            start=False, stop=True, skip_group_check=True,
                        )
                    ot = tmp.tile([P, 4 * OUT], f32, name="ot", tag="ot")
                    nc.vector.tensor_copy(ot[:], po[:])
                    nc.sync.dma_start(
                        out=outp[c * cw_y:(c + 1) * cw_y, :].rearrange(
                            "(t p) f -> p t f", p=P),
                        in_=ot.rearrange("p (t f) -> p t f", f=OUT),
                    )
            if layer < L - 1:
                emit_gather(hm1, 1, rest_order=rest2)


def build_full(n_cores=N_CORES, n=N, adj_dtype="fp8", h_dtype="fp8",
               use_doublerow=True, num_devices=None, with_collective=True,
               repeats=1, b_res=B_RES, ring1=RING1, ring2=RING2):
    """Build + compile the full SPMD Bass module (one program, 8 cores)."""
    import concourse.bacc as bacc
    import concourse.mybir as mybir
    import concourse.tile as tile

    dt = mybir.dt
    f32, bf = dt.float32, dt.bfloat16
    adt = dt.float8e4
    rpc = n // n_cores
    if num_devices is None:
        num_devices = n_cores

    nc = bacc.Bacc("TRN2", target_bir_lowering=False, debug=False,
                   num_devices=num_devices)
    ins = {
        "adjt": nc.dram_tensor("adjt", [n, rpc], adt, kind="ExternalInput").ap(),
        "zsut": nc.dram_tensor("zsut", [P, rpc], bf, kind="ExternalInput").ap(),
        "zat": nc.dram_tensor("zat", [D, rpc], bf, kind="ExternalInput").ap(),
        "wbd": nc.dram_tensor("wbd", [P, L * P], bf, kind="ExternalInput").ap(),
        "wlsu": nc.dram_tensor("wlsu", [P, OUT], bf, kind="ExternalInput").ap(),
        "wlza": nc.dram_tensor("wlza", [D, OUT], bf, kind="ExternalInput").ap(),
        "blr": nc.dram_tensor("blr", [1, OUT], bf, kind="ExternalInput").ap(),
        "bias": nc.dram_tensor("bias", [P, L], f32, kind="ExternalInput").ap(),
    }
    outs = {
        "outp": nc.dram_tensor("outp", [rpc, OUT], f32, kind="ExternalOutput").ap(),
    }
    with tile.TileContext(nc) as tc:
        for _ in range(repeats):
            build_kernel_body(tc, ins, outs, n_cores=n_cores, n=n,
                              with_collective=with_collective, b_res=b_res,
                              ring1=ring1, ring2=ring2)
    nc.compile()
    return nc


def prep_inputs(z, adj, Ws, bs, Wl, bl, n_cores=N_CORES, n=N, adj_dtype="fp8"):
    """Host-side sharding: build the per-core input maps."""
    rpc = n // n_cores
    z = np.asarray(z, dtype=np.float32)
    adj = np.asarray(adj, dtype=np.float32)
    Ws = np.asarray(Ws, dtype=np.float32)
    bs = np.asarray(bs, dtype=np.float32)
    Wl = np.asarray(Wl, dtype=np.float32)
    bl = np.asarray(bl, dtype=np.float32)

    adjt = (adj.T * np.float32(n)).astype(FP8)           # [n, n] in [0,1)
    zsut = np.ascontiguousarray(z[:, :2 * D].T).astype(BF16)  # [128, n]
    zat = np.ascontiguousarray(z[:, 2 * D:].T).astype(BF16)   # [64, n]

    wbd = np.zeros((P, L * P), dtype=np.float32)
    for l in range(L):
        wbd[:D, l * P:l * P + D] = Ws[l]
        wbd[D:, l * P + D:(l + 1) * P] = Ws[l]
    wbd = wbd.astype(BF16)
    bias = np.stack([np.concatenate([bs[l], bs[l]]) for l in range(L)],
                    axis=1).astype(np.float32)           # [128, L]
    wlsu = np.ascontiguousarray(Wl[:2 * D]).astype(BF16)
    wlza = np.ascontiguousarray(Wl[2 * D:]).astype(BF16)
    blr = np.ascontiguousarray(bl.reshape(1, OUT)).astype(BF16)

    in_maps = []
    for m in range(n_cores):
        sl = slice(m * rpc, (m + 1) * rpc)
        in_maps.append({
            "adjt": np.ascontiguousarray(adjt[:, sl]),
            "zsut": np.ascontiguousarray(zsut[:, sl]),
            "zat": np.ascontiguousarray(zat[:, sl]),
            "wbd": wbd,
            "wlsu": wlsu,
            "wlza": wlza,
            "blr": blr,
            "bias": bias,
        })
    return in_maps


_NC_CACHE = {}
ADJ_DTYPE = "fp8"
H_DTYPE = "fp8"
USE_DOUBLEROW = True


def kernel(z, adj, Ws, bs, Wl, bl):
    """Full-input entry point: shard, run on 8 NeuronCores, gather."""
    from concourse.bass_utils import run_bass_kernel_spmd

    if "nc" not in _NC_CACHE:
        _NC_CACHE["nc"] = build_full()
    nc = _NC_CACHE["nc"]

    in_maps = prep_inputs(z, adj, Ws, bs, Wl, bl)
    res = run_bass_kernel_spmd(nc, in_maps, core_ids=list(range(N_CORES)))
    out = np.concatenate(
        [res.results[m]["outp"] for m in range(N_CORES)], axis=0
    ).astype(np.float32)
    return out


# revision 17
# speedup vs baseline: 1.1799x; 1.0280x over previous
"""Trainium2 Bass kernel for a 2-layer dual-branch GCN (nn_ATACGCN).

reference:
    zs, zu, za = split(z)
    ys = elu(adj @ (elu(zs) @ W0) + b0); ys = elu(adj @ (ys @ W1) + b1)
    yu = elu(adj @ (elu(zu) @ W0) + b0); yu = elu(adj @ (yu @ W1) + b1)
    out = concat(ys, yu, za) @ Wl + bl

Strategy: 1D row-shard of the node dimension across 8 NeuronCores. Both
branches share weights, so they are fused into one 128-wide feature block
(block-diagonal W). Each core computes Y^T = H^T @ adjT in PSUM, streaming
its [16384, 2048] slab of adj^T (fp8, host-scaled by N) from HBM.

The kernel is DMA-bound (adj streaming), so the first B_RES kb-blocks of the
adj slab are kept RESIDENT in SBUF across both GCN layers -- layer 2 only
re-streams the non-resident remainder. Stage A (input elu + H0 = E @ W0) is
sharded across cores and AllGathered, like the inter-layer exchange, instead
of being computed redundantly on every core. h-path DMAs ride the ACT HWDGE
ring so they never queue behind bulk adj DMAs on the SP ring.

elu(x) is composed as min(exp(x) - 1, max(x, 0)); exp/relu run on ACT with
the fp8 descale (1/n) and layer bias folded into the activation's
scale/bias operands.
"""

import numpy as np
import ml_dtypes

BF16 = ml_dtypes.bfloat16
FP8 = ml_dtypes.float8_e4m3

# Problem constants (hardcoded per harness contract).
N = 16384      # nodes
D = 64         # per-branch width
OUT = 64       # output width
L = 2          # gcn layers
N_CORES = 8
P = 128        # SBUF partitions
RPC = N // N_CORES          # rows (nodes) per core

# Tunables
B_RES = 32                  # resident kb-blocks (of kt//2 = 64); 4 KiB/part each
RES_GRP = 4                 # kb-blocks per resident group DMA (2 MiB DMAs)
SG = 2                      # kb-blocks per streamed slab DMA (1 MiB DMAs)
RING1 = 2                   # L1 streaming slab ring depth (units of SG kbs)
RING2 = 2                   # L2 streaming slab ring depth (fresh tag: prefetch
                            # across the inter-layer collective)
CW_SUB = 512                # stage-A / elu subchunk width


def build_kernel_body(tc, ins, outs, n_cores=N_CORES, n=N, with_collective=True,
                      b_res=B_RES, ring1=RING1, ring2=RING2):
    """Emit the per-core Tile program (fp8 adj + fp8 H, DoubleRow matmuls).

    ins/outs: dicts name -> bass.AP of the DRAM I/O tensors:
      adjt [n, rpc] fp8 (adj.T * n, host-scaled), zsut [128, rpc] bf16,
      zat [64, rpc] bf16, wbd [128, 2*128] bf16, wlsu [128, 64] bf16,
      wlza [64, 64] bf16, blr [1, 64] bf16, bias [128, 2] f32
      ->  outp [rpc, 64] f32
    """
    import concourse.mybir as mybir

    nc = tc.nc
    dt = mybir.dt
    f32, bf = dt.float32, dt.bfloat16
    AF = mybir.ActivationFunctionType
    ALU = mybir.AluOpType
    adt = dt.float8e4
    hdt = dt.float8e4
    inv_n = 1.0 / n

    rpc = n // n_cores
    kt = n // P                  # 128 k-tiles
    nkb = kt // 2                # 64 kb-blocks (DoubleRow: 2 k-tiles each)
    t_pc = rpc // P              # 16 node tiles per core
    cw_y = 512                   # PSUM chunk width (one f32 bank)
    nch_y = rpc // cw_y          # 4
    n_sub = rpc // CW_SUB        # stage-A subchunks
    assert b_res % RES_GRP == 0
    n_res_grp = b_res // RES_GRP

    adjt = ins["adjt"]
    zsut = ins["zsut"]
    zat = ins["zat"]
    wbd, wlsu, wlza = ins["wbd"], ins["wlsu"], ins["wlza"]
    blr, bias = ins["blr"], ins["bias"]
    outp = outs["outp"]

    # DRAM view: q = global k-tile index (0..127).
    adjq = adjt.rearrange("(q p) m -> p q m", p=P)   # [128, 128, rpc]

    with (
        tc.tile_pool(name="consts", bufs=1) as consts,
        tc.tile_pool(name="respool", bufs=1) as respool,
        tc.tile_pool(name="hpool", bufs=1) as hpool,
        tc.tile_pool(name="adjp", bufs=ring1) as adjp,
        tc.tile_pool(name="adjp2", bufs=ring2) as adjp2,
        tc.tile_pool(name="tmp", bufs=2) as tmp,
        tc.tile_pool(name="xp", bufs=1) as xp,
        tc.tile_pool(name="ps", bufs=1, space="PSUM") as ps,
        tc.tile_pool(name="dram", bufs=1, space="DRAM") as dram,
    ):
        # ---- constants to SBUF ----
        wbd_sb = consts.tile([P, L * P], bf, name="wbd_sb")
        nc.scalar.dma_start(out=wbd_sb[:], in_=wbd[:])
        wlsu_sb = consts.tile([P, OUT], bf, name="wlsu_sb")
        nc.scalar.dma_start(out=wlsu_sb[:], in_=wlsu[:])
        wlza_sb = consts.tile([D, OUT], bf, name="wlza_sb")
        nc.scalar.dma_start(out=wlza_sb[:], in_=wlza[:])
        blr_sb = consts.tile([1, OUT], bf, name="blr_sb")
        nc.scalar.dma_start(out=blr_sb[:], in_=blr[:])
        bias_sb = consts.tile([P, L], f32, name="bias_sb")
        nc.scalar.dma_start(out=bias_sb[:], in_=bias[:])
        zat_sb = consts.tile([D, rpc], bf, name="zat_sb")
        nc.sync.dma_start(out=zat_sb[:], in_=zat[:])
        ones_sb = consts.tile([1, P], bf, name="ones_sb")
        nc.vector.memset(ones_sb[:], 1.0)

        # Persistent H tile (shared between layers; 16 KiB/partition).
        # Layout: h[p, q*128 + f] = H[node q*128+p, f].
        h = hpool.tile([P, n], hdt, name="h", tag="h")
        hq = h.rearrange("p (q f) -> p q f", f=P)

        def emit_gather(hm, layer, rest_order=None):
            """AllGather local node-block features hm [P, rpc] -> full h.

            g_in rows are ordered (partition, tile) so that both the g_in
            write and the h restage run with >=2 KiB-contiguous descriptors
            per partition (128 B descriptors otherwise -- far below the
            512 B full-rate SDMA minimum).
            """
            g_in = dram.tile([rpc, P], hdt, name=f"g_in{layer}")
            nc.scalar.dma_start(
                out=g_in.rearrange("(p t) f -> p (t f)", p=P),
                in_=hm[:],
            )
            if with_collective and n_cores > 1:
                g_out = dram.tile([n, P], hdt, name=f"g_out{layer}",
                                  addr_space="Shared")
                nc.gpsimd.collective_compute(
                    "AllGather",
                    mybir.AluOpType.bypass,
                    replica_groups=[list(range(n_cores))],
                    ins=[g_in.opt()],
                    outs=[g_out.opt()],
                )
            else:
                # cost-model-only path (TimelineSim): same DMA pattern minus
                # the collective. Numerically invalid for other cores' tiles.
                g_out = dram.tile([n, P], hdt, name=f"g_out{layer}")
                nc.scalar.dma_start(out=g_out[:rpc, :], in_=g_in[:])
            # g_out row m*rpc + p*t_pc + t holds H[node m*rpc + t*128 + p, :].
            # h free-dim block m*2048 + t*128 + f <- g_out[m, p] run (t f).
            gm = g_out.rearrange("(m p w) f -> p m (w f)", m=n_cores, p=P)
            hm_view = h.rearrange("p (m w) -> p m w", m=n_cores)
            for g in (rest_order or range(n_cores)):
                nc.scalar.dma_start(
                    out=hm_view[:, g:g + 1, :],
                    in_=gm[:, g:g + 1, :],
                )

        # ---- stage A (redundant on every core): H0 = elu(zsu) @ W0bd ----
        # Full recompute instead of shard+AllGather: one fewer collective
        # (~36 us on HW) for ~11 us of extra zsut streaming, fully overlapped
        # with the resident-adj loads at startup.
        zbig = n // 8
        for ch in range(n // zbig):
            zch = tmp.tile([P, zbig], bf, name="zch", tag="zch")
            nc.scalar.dma_start(
                out=zch[:], in_=zsut[:, ch * zbig:(ch + 1) * zbig])
            for sc in range(zbig // CW_SUB):
                sl = slice(sc * CW_SUB, (sc + 1) * CW_SUB)
                hsl = slice(ch * zbig + sc * CW_SUB,
                            ch * zbig + (sc + 1) * CW_SUB)
                e_t = tmp.tile([P, CW_SUB], f32, name="e_t", tag="e_t")
                nc.scalar.activation(e_t[:], zch[:, sl], AF.Exp)
                m_t = tmp.tile([P, CW_SUB], f32, name="m_t", tag="m_t")
                nc.scalar.activation(m_t[:], zch[:, sl], AF.Relu)
                x0 = tmp.tile([P, CW_SUB], bf, name="x0", tag="x0")
                nc.vector.scalar_tensor_tensor(
                    x0[:], e_t[:], -1.0, m_t[:], op0=ALU.add, op1=ALU.min
                )
                ph = ps.tile([P, CW_SUB], f32, name="ph", tag="ph", bufs=2)
                for t4 in range(CW_SUB // P):
                    nc.tensor.matmul(
                        ph[:, t4 * P:(t4 + 1) * P],
                        lhsT=x0[:, t4 * P:(t4 + 1) * P], rhs=wbd_sb[:, 0:P],
                        start=True, stop=True,
                    )
                nc.vector.tensor_copy(h[:, hsl], ph[:])

        # ---- resident adj groups (loaded once, used by both layers) ----
        res_tiles = [
            respool.tile([P, RES_GRP * 2, rpc], adt, name=f"res{g}",
                         tag=f"res{g}")
            for g in range(n_res_grp)
        ]
        # L1's last ring1 slab units stay valid in SBUF; L2 reuses them.
        l1_tail = {}     # unit kb0 -> slab tile

        # Work units: ("res", g) = resident group (RES_GRP kbs, no DMA in L2),
        # ("slab", kb0) = streamed slab of SG kbs, ("tail", kb0) = L1-tail
        # slab reused in L2 without DMA.
        s_units = list(range(b_res, nkb, SG))
        tail_units = s_units[len(s_units) - ring1:]
        res_units = [("res", g) for g in range(n_res_grp)]

        # L1: residents first (PE races ahead while their 2 MiB loads and the
        # stream pipeline fill DMA), streamed units last (DMA-paced finish).
        l1_order = res_units + [("slab", kb0) for kb0 in s_units]

        # L2: ring2-prefetched stream units first (release stream slots the
        # moment h lands), then remaining stream units Bresenham-interleaved
        # with no-DMA units (residents + reused L1-tail slabs) so PE
        # slot-release tracks DMA pace and resident work fills the DMA-bound
        # phase instead of forming a serial PE tail.
        new_units = [kb0 for kb0 in s_units if kb0 not in tail_units]
        nodma = res_units + [("tail", kb0) for kb0 in tail_units]
        nodma_kbs = sum(RES_GRP if u[0] == "res" else SG for u in nodma)
        l2_order = [("slab", kb0) for kb0 in new_units[:ring2]]
        rest_units = new_units[ring2:]
        ratio = nodma_kbs / max(1, len(rest_units))
        acc, ri = 0.0, 0
        for kb0 in rest_units:
            l2_order.append(("slab", kb0))
            acc += ratio
            while acc >= 1.0 and ri < len(nodma):
                u = nodma[ri]
                l2_order.append(u)
                acc -= RES_GRP if u[0] == "res" else SG
                ri += 1
        l2_order.extend(nodma[ri:])

        def unit_kbs(u):
            kind, v = u
            if kind == "res":
                return list(range(v * RES_GRP, (v + 1) * RES_GRP))
            return list(range(v, v + SG))

        assert sorted(kb for u in l2_order for kb in unit_kbs(u)) == \
            list(range(nkb))
        # restage-DMA order: deliver h m-groups in L2 consumption order
        rest2, seen = [], set()
        for u in l2_order:
            for kb in unit_kbs(u):
                g = kb // (nkb // n_cores)
                if g not in seen:
                    seen.add(g)
                    rest2.append(g)
        unit_orders = [l1_order, l2_order]

        for layer in range(L):
            # ---- big matmul: Y^T[feat, local nodes] = H^T @ adjT ----
            psy = [
                ps.tile([P, cw_y], f32, name=f"psy{c}", tag=f"psy{c}", bufs=1)
                for c in range(nch_y)
            ]
            pos = 0
            for u in unit_orders[layer]:
                kind, v = u
                if kind == "res":
                    if layer == 0:
                        nc.sync.dma_start(
                            out=res_tiles[v][:],
                            in_=adjq[:, v * RES_GRP * 2:(v + 1) * RES_GRP * 2, :],
                        )
                    rhs_t = res_tiles[v]
                    kb0 = v * RES_GRP
                elif kind == "tail" or (layer == 1 and v in l1_tail):
                    rhs_t = l1_tail[v]
                    kb0 = v
                else:
                    pool = adjp if layer == 0 else adjp2
                    slab = pool.tile([P, 2 * SG, rpc], adt, name="slab",
                                     tag=f"slab{layer}")
                    nc.sync.dma_start(
                        out=slab[:], in_=adjq[:, 2 * v:2 * (v + SG), :]
                    )
                    if layer == 0 and v in tail_units:
                        l1_tail[v] = slab
                    rhs_t = slab
                    kb0 = v
                for i, kb in enumerate(unit_kbs(u)):
                    rhs3 = rhs_t[:, 2 * i:2 * i + 2, :]
                    lhsT = hq[:, 2 * kb:2 * kb + 2, :]
                    for c in range(nch_y):
                        nc.tensor.matmul(
                            psy[c][:],
                            lhsT=lhsT,
                            rhs=rhs3[:, :, c * cw_y:(c + 1) * cw_y],
                            start=(pos == 0), stop=(pos == nkb - 1),
                            perf_mode=mybir.MatmulPerfMode.DoubleRow,
                        )
                    pos += 1

            # ---- per-chunk: X^T = elu(Y^T/n + b), then H1 | final out ----
            b_ap = bias_sb[:, layer:layer + 1]
            hm1 = None
            if layer < L - 1:
                hm1 = xp.tile([P, rpc], hdt, name="hm", tag="hm")
            for c in range(nch_y):
                e_t = tmp.tile([P, cw_y], f32, name="e_t", tag="e_t")
                nc.scalar.activation(e_t[:], psy[c][:], AF.Exp,
                                     bias=b_ap, scale=inv_n)
                m_t = tmp.tile([P, cw_y], f32, name="m_t", tag="m_t")
                nc.scalar.activation(m_t[:], psy[c][:], AF.Relu,
                                     bias=b_ap, scale=inv_n)
                xc = tmp.tile([P, cw_y], bf, name="xc", tag="xc")
                nc.vector.scalar_tensor_tensor(
                    xc[:], e_t[:], -1.0, m_t[:], op0=ALU.add, op1=ALU.min,
                )
                if layer < L - 1:
                    # H1 chunk: 4 node tiles -> one PSUM bank -> hm1
                    ph = ps.tile([P, 4 * P], f32, name="ph", tag="ph", bufs=2)
                    for t4 in range(4):
                        nc.tensor.matmul(
                            ph[:, t4 * P:(t4 + 1) * P],
                            lhsT=xc[:, t4 * P:(t4 + 1) * P],
                            rhs=wbd_sb[:, P:2 * P], start=True, stop=True,
                        )
                    nc.vector.tensor_copy(
                        hm1[:, c * cw_y:(c + 1) * cw_y], ph[:])
                else:
                    # final: out = [ys yu] @ Wl[:128] + za @ Wl[128:] + bl
                    po = ps.tile([P, 4 * OUT], f32, name="po", tag="po",
                                 bufs=2)
                    for t4 in range(4):
                        t = c * 4 + t4
                        osl = slice(t4 * OUT, (t4 + 1) * OUT)
                        nc.tensor.matmul(
                            po[:, osl], lhsT=xc[:, t4 * P:(t4 + 1) * P],
                            rhs=wlsu_sb[:],
                            start=True, stop=False, skip_group_check=True,
                        )
                        nc.tensor.matmul(
                            po[:, osl], lhsT=zat_sb[:, t * P:(t + 1) * P],
                            rhs=wlza_sb[:],
                            start=False, stop=False, skip_group_check=True,
                        )
                        nc.tensor.matmul(
                            po[:, osl], lhsT=ones_sb[:], rhs=blr_sb[:],
                            start=False, stop=True, skip_group_check=True,
                        )
                    ot = tmp.tile([P, 4 * OUT], f32, name="ot", tag="ot")
                    nc.vector.tensor_copy(ot[:], po[:])
                    nc.sync.dma_start(
                        out=outp[c * cw_y:(c + 1) * cw_y, :].rearrange(
                            "(t p) f -> p t f", p=P),
                        in_=ot.rearrange("p (t f) -> p t f", f=OUT),
                    )
            if layer < L - 1:
                emit_gather(hm1, 1, rest_order=rest2)


def build_full(n_cores=N_CORES, n=N, adj_dtype="fp8", h_dtype="fp8",
               use_doublerow=True, num_devices=None, with_collective=True,
               repeats=1, b_res=B_RES, ring1=RING1, ring2=RING2):
    """Build + compile the full SPMD Bass module (one program, 8 cores)."""
    import concourse.bacc as bacc
    import concourse.mybir as mybir
    import concourse.tile as tile

    dt = mybir.dt
    f32, bf = dt.float32, dt.bfloat16
    adt = dt.float8e4
    rpc = n // n_cores
    if num_devices is None:
        num_devices = n_cores

    nc = bacc.Bacc("TRN2", target_bir_lowering=False, debug=False,
                   num_devices=num_devices)
    ins = {
        "adjt": nc.dram_tensor("adjt", [n, rpc], adt, kind="ExternalInput").ap(),
        "zsut": nc.dram_tensor("zsut", [P, n], bf, kind="ExternalInput").ap(),
        "zat": nc.dram_tensor("zat", [D, rpc], bf, kind="ExternalInput").ap(),
        "wbd": nc.dram_tensor("wbd", [P, L * P], bf, kind="ExternalInput").ap(),
        "wlsu": nc.dram_tensor("wlsu", [P, OUT], bf, kind="ExternalInput").ap(),
        "wlza": nc.dram_tensor("wlza", [D, OUT], bf, kind="ExternalInput").ap(),
        "blr": nc.dram_tensor("blr", [1, OUT], bf, kind="ExternalInput").ap(),
        "bias": nc.dram_tensor("bias", [P, L], f32, kind="ExternalInput").ap(),
    }
    outs = {
        "outp": nc.dram_tensor("outp", [rpc, OUT], f32, kind="ExternalOutput").ap(),
    }
    with tile.TileContext(nc) as tc:
        for _ in range(repeats):
            build_kernel_body(tc, ins, outs, n_cores=n_cores, n=n,
                              with_collective=with_collective, b_res=b_res,
                              ring1=ring1, ring2=ring2)
    nc.compile()
    return nc


def prep_inputs(z, adj, Ws, bs, Wl, bl, n_cores=N_CORES, n=N, adj_dtype="fp8"):
    """Host-side sharding: build the per-core input maps."""
    rpc = n // n_cores
    z = np.asarray(z, dtype=np.float32)
    adj = np.asarray(adj, dtype=np.float32)
    Ws = np.asarray(Ws, dtype=np.float32)
    bs = np.asarray(bs, dtype=np.float32)
    Wl = np.asarray(Wl, dtype=np.float32)
    bl = np.asarray(bl, dtype=np.float32)

    adjt = (adj.T * np.float32(n)).astype(FP8)           # [n, n] in [0,1)
    zsut = np.ascontiguousarray(z[:, :2 * D].T).astype(BF16)  # [128, n]
    zat = np.ascontiguousarray(z[:, 2 * D:].T).astype(BF16)   # [64, n]

    wbd = np.zeros((P, L * P), dtype=np.float32)
    for l in range(L):
        wbd[:D, l * P:l * P + D] = Ws[l]
        wbd[D:, l * P + D:(l + 1) * P] = Ws[l]
    wbd = wbd.astype(BF16)
    bias = np.stack([np.concatenate([bs[l], bs[l]]) for l in range(L)],
                    axis=1).astype(np.float32)           # [128, L]
    wlsu = np.ascontiguousarray(Wl[:2 * D]).astype(BF16)
    wlza = np.ascontiguousarray(Wl[2 * D:]).astype(BF16)
    blr = np.ascontiguousarray(bl.reshape(1, OUT)).astype(BF16)

    in_maps = []
    for m in range(n_cores):
        sl = slice(m * rpc, (m + 1) * rpc)
        in_maps.append({
            "adjt": np.ascontiguousarray(adjt[:, sl]),
            "zsut": zsut,
            "zat": np.ascontiguousarray(zat[:, sl]),
            "wbd": wbd,
            "wlsu": wlsu,
            "wlza": wlza,
            "blr": blr,
            "bias": bias,
        })
    return in_maps


_NC_CACHE = {}
ADJ_DTYPE = "fp8"
H_DTYPE = "fp8"
USE_DOUBLEROW = True


def kernel(z, adj, Ws, bs, Wl, bl):
    """Full-input entry point: shard, run on 8 NeuronCores, gather."""
    from concourse.bass_utils import run_bass_kernel_spmd

    if "nc" not in _NC_CACHE:
        _NC_CACHE["nc"] = build_full()
    nc = _NC_CACHE["nc"]

    in_maps = prep_inputs(z, adj, Ws, bs, Wl, bl)
    res = run_bass_kernel_spmd(nc, in_maps, core_ids=list(range(N_CORES)))
    out = np.concatenate(
        [res.results[m]["outp"] for m in range(N_CORES)], axis=0
    ).astype(np.float32)
    return out
